# revision 44
# baseline (speedup 1.0000x reference)
"""CPSF Memcell Autoencoder on 8 Trainium2 cores — pure data parallel.

Per-core: 1 image [3,256,256]. Encoder (2 conv paths, bf16 staging) ->
deferred batched softmax over 32 slots -> global delta-rule V update via
AllGather of per-core dV -> deconv decoder (psum-packed conv3) back to
[3,256,256].
"""
import sys
sys.path.insert(0, '/opt/trn_rl_repo')
import numpy as np
import ml_dtypes
import concourse.bass as bass
import concourse.bacc as bacc
import concourse.mybir as mybir
import concourse.tile as tile
from concourse import masks
from concourse.bass_utils import run_bass_kernel_spmd

f32 = mybir.dt.float32
f32r = mybir.dt.float32r
bf16 = mybir.dt.bfloat16
AF = mybir.ActivationFunctionType
ALU = mybir.AluOpType
BF = ml_dtypes.bfloat16

N_CORES = 8
N, M, S = 16, 32, 128
ALPHA = 1e-06
H = W = 256          # full res
NSTRIP = 8           # encoder strips
SR = 32              # conv1 out rows per strip (plus 2 halo rows)
RW = 258             # padded row width in a1 buffers

_cache = {}


def _mk_ap(tile_ap, offset, dims):
    """Manual AP: partition dim from tile_ap, then free dims [[step,count],...]."""
    part = list(tile_ap.ap[0])
    return bass.AP(tile_ap.tensor, offset, [part] + [list(d) for d in dims])


def _build(sc_sim=False):
    nc = bacc.Bacc("TRN2", target_bir_lowering=False)

    # ---------------- DRAM I/O ----------------
    x_d = nc.dram_tensor("x", [3, H, W], f32r, kind="ExternalInput")
    w1s_d = nc.dram_tensor("w1s", [27, 128], bf16, kind="ExternalInput")
    w1n4_d = nc.dram_tensor("w1n4", [108, 64], bf16, kind="ExternalInput")
    w2s_d = nc.dram_tensor("w2s", [128, 9 * 128], bf16, kind="ExternalInput")
    w2nA_d = nc.dram_tensor("w2nA", [128, 16], bf16, kind="ExternalInput")
    w2nB_d = nc.dram_tensor("w2nB", [16, 16], bf16, kind="ExternalInput")
    ckt_d = nc.dram_tensor("ckt", [16, 32], f32r, kind="ExternalInput")
    v_d = nc.dram_tensor("vmat", [32, 128], f32r, kind="ExternalInput")
    decw_d = nc.dram_tensor("decw", [128, 9 * 12], f32r, kind="ExternalInput")
    lhs48_d = nc.dram_tensor("lhs48", [96, 128], f32r, kind="ExternalInput")
    b1s_d = nc.dram_tensor("b1s", [128, 1], f32, kind="ExternalInput")
    b1n_d = nc.dram_tensor("b1n", [64, 1], f32, kind="ExternalInput")
    b2s_d = nc.dram_tensor("b2s", [128, 1], f32, kind="ExternalInput")
    b2n_d = nc.dram_tensor("b2n", [16, 1], f32, kind="ExternalInput")
    bdec_d = nc.dram_tensor("bdec", [12, 1], f32, kind="ExternalInput")
    b3r_d = nc.dram_tensor("b3r", [128, 1], f32, kind="ExternalInput")
    zer_d = nc.dram_tensor("zer", [128, 1024], f32r, kind="ExternalInput")
    zerbf_d = nc.dram_tensor("zerbf", [128, 512], bf16, kind="ExternalInput")
    zer27_d = nc.dram_tensor("zer27", [27, 34 * 256], bf16, kind="ExternalInput")
    out_d = nc.dram_tensor("out", [3, H, W], f32, kind="ExternalOutput")

    with tile.TileContext(nc) as tc:
        with (
            tc.tile_pool(name="pconst", bufs=1) as pc,
            tc.tile_pool(name="ppersist", bufs=1) as pp,
            tc.tile_pool(name="pdram", bufs=1, space="DRAM") as pdram,
        ):
            # ------- constants -------
            w1s = pc.tile([27, 128], bf16); nc.sync.dma_start(w1s[:], w1s_d[:])
            w1n4 = pc.tile([108, 64], bf16); nc.sync.dma_start(w1n4[:], w1n4_d[:])
            w2s = pc.tile([128, 9 * 128], bf16)
            nc.sync.dma_start(w2s[:], w2s_d[:])
            w2nA = pc.tile([128, 16], bf16); nc.sync.dma_start(w2nA[:], w2nA_d[:])
            w2nB = pc.tile([16, 16], bf16); nc.sync.dma_start(w2nB[:], w2nB_d[:])
            ckt = pc.tile([16, 32], f32r); nc.sync.dma_start(ckt[:], ckt_d[:])
            vmat = pc.tile([32, 128], f32r); nc.sync.dma_start(vmat[:], v_d[:])
            decw = pc.tile([128, 9 * 12], f32r)
            nc.sync.dma_start(decw[:], decw_d[:])
            lhs48 = pc.tile([96, 128], f32r); nc.sync.dma_start(lhs48[:], lhs48_d[:])
            b1s = pc.tile([128, 1], f32); nc.sync.dma_start(b1s[:], b1s_d[:])
            b1n = pc.tile([64, 1], f32); nc.sync.dma_start(b1n[:], b1n_d[:])
            b2s = pc.tile([128, 1], f32); nc.sync.dma_start(b2s[:], b2s_d[:])
            b2n = pc.tile([16, 1], f32); nc.sync.dma_start(b2n[:], b2n_d[:])
            bdec = pc.tile([12, 1], f32); nc.sync.dma_start(bdec[:], bdec_d[:])
            b3r = pc.tile([128, 1], f32); nc.sync.dma_start(b3r[:], b3r_d[:])
            ident = pc.tile([128, 128], f32)
            masks.make_identity(nc, ident[:])
            identb = pc.tile([128, 128], bf16)
            nc.vector.tensor_copy(identb[:], ident[:])

            # ------- persistent across phases -------
            w_bf = pp.tile([128, 32 * 128], bf16)       # token-major softmax w
            vnew_bf = pp.tile([32, 128], bf16)

            # =====================  ENCODER  =====================
            with tc.tile_pool(name="pEnc", bufs=1) as pEo:
                tstT_all = pEo.tile([128, 16384], bf16)   # t*^T, token-major
                logit_all = pEo.tile([128, 4096], f32)

                with (
                    tc.tile_pool(name="pE", bufs=1) as pE,
                    tc.tile_pool(name="pEd", bufs=1) as pEd,
                    tc.tile_pool(name="pst", bufs=2) as pst,
                    tc.tile_pool(name="psA", bufs=1, space="PSUM") as psA,
                    tc.tile_pool(name="psB", bufs=1, space="PSUM") as psB,
                    tc.tile_pool(name="psC2", bufs=2, space="PSUM") as psC2,
                ):
                    NR = 32          # full-res rows per strip
                    R1 = NR + 2      # conv1 rows incl halo
                    NCH = R1 * 256 // 512          # conv1 px chunks (2 rows)
                    NQ = (NR // 2) * 128 // 512    # conv2 px chunks of 512
                    for s in range(NSTRIP):
                        y0 = NR * s
                        # ---- im2col for conv1 ----
                        im1 = pEd.tile([27, R1 * 256], bf16, tag="im1", bufs=2)
                        nc.sync.dma_start(im1[:], zer27_d[:, 0:R1 * 256])
                        for ky in range(3):
                            for kx in range(3):
                                t = ky * 3 + kx
                                r_lo = max(0, 2 - y0 - ky)
                                r_hi = min(R1, 258 - y0 - ky)
                                c_lo = max(0, 1 - kx)
                                c_hi = min(256, 257 - kx)
                                nc.gpsimd.dma_start(
                                    im1[3 * t:3 * t + 3, :]
                                    .rearrange("p (r c) -> p r c", r=R1)
                                    [:, r_lo:r_hi, c_lo:c_hi],
                                    x_d[0:3,
                                        y0 - 2 + r_lo + ky: y0 - 2 + r_hi + ky,
                                        c_lo + kx - 1: c_hi + kx - 1])

                        # ---- conv1 wide + narrow ----
                        a1s = pEd.tile([128, R1 * RW], bf16, tag="a1s", bufs=2)
                        # a1n flat, col-deinterleaved: row lr at [lr*260,..):
                        # evens (cx=2e) at +e, odds (cx=2j+1) at +130+j
                        a1n = pEd.tile([16, R1 * 260], bf16, tag="a1n")
                        # narrow conv1, G4: im1n4[27g+k, 512j] = im1[k, 512(4j+g)]
                        im1n4 = pE.tile([108, 5 * 512], bf16, tag="im1n4",
                                        bufs=2)
                        # a1n_g4[16g+c, 520j + u] = a1n-layout chunk 4j+g
                        a1ng = pE.tile([64, 5 * 520], bf16, tag="a1ng", bufs=2)
                        for g4 in range(4):
                            cnt = 5 if g4 == 0 else 4
                            nc.sync.dma_start(
                                im1n4[27 * g4:27 * g4 + 27, 0:512 * cnt]
                                .rearrange("p (j c) -> p j c", c=512),
                                _mk_ap(im1[:], 512 * g4,
                                       [[2048, cnt], [1, 512]]))
                        for j in range(5):
                            npart = 64 if j < 4 else 16
                            kk = 108 if j < 4 else 27
                            c1n = psB.tile([128, 512], f32, tag="c1s",
                                           bufs=2)
                            nc.tensor.matmul(
                                c1n[0:npart, :], w1n4[0:kk, 0:npart],
                                im1n4[0:kk, 512 * j:512 * (j + 1)],
                                start=True, stop=True)
                            nc.scalar.activation(
                                _mk_ap(a1ng[0:npart, :], 520 * j,
                                       [[260, 2], [1, 128], [130, 2]]),
                                c1n[0:npart, :]
                                .rearrange("p (r e two) -> p r e two",
                                           r=2, two=2),
                                AF.Silu, bias=b1n[0:npart, :])
                        for g4 in range(4):
                            cnt = 5 if g4 == 0 else 4
                            nc.sync.dma_start(
                                _mk_ap(a1n[:], 520 * g4,
                                       [[2080, cnt], [1, 520]]),
                                a1ng[16 * g4:16 * g4 + 16, 0:520 * cnt]
                                .rearrange("p (j c) -> p j c", c=520))
                        if s == 0:      # conv2 zero-pad at image top
                            nc.sync.dma_start(a1n[:, 0:260], zerbf_d[0:16, 0:260])
                        if s == NSTRIP - 1:  # bottom: lr = R1-1
                            nc.sync.dma_start(
                                a1n[:, (R1 - 1) * 260:R1 * 260],
                                zerbf_d[0:16, 0:260])

                        # ---- im2col for conv2 narrow (stride 2) ----
                        NPX2 = (NR // 2) * 128      # conv2 out px per strip
                        im2A = pE.tile([128, NPX2], bf16, tag="im2A", bufs=2)
                        im2B = pE.tile([16, NPX2], bf16, tag="im2B", bufs=2)
                        for ky in range(3):
                            for kx in range(3):
                                t = ky * 3 + kx
                                dstt = im2A if t < 8 else im2B
                                prow = 16 * t if t < 8 else 0
                                dst = dstt[prow:prow + 16, :] \
                                    .rearrange("p (r c) -> p r c", c=128)
                                eng2 = nc.sync if t % 2 == 0 else nc.gpsimd
                                if kx == 1:    # evens e=ox
                                    src = _mk_ap(a1n[:], ky * 260,
                                                 [[520, NR // 2], [1, 128]])
                                    eng2.dma_start(dst[:, :, :].opt(), src)
                                elif kx == 2:  # odds j=ox
                                    src = _mk_ap(a1n[:], ky * 260 + 130,
                                                 [[520, NR // 2], [1, 128]])
                                    eng2.dma_start(dst[:, :, :].opt(), src)
                                else:          # kx=0: odds j=ox-1; col ox=0 zero
                                    src = _mk_ap(a1n[:], ky * 260 + 130,
                                                 [[520, NR // 2], [1, 127]])
                                    eng2.dma_start(dst[:, :, 1:128].opt(), src)
                                    eng2.dma_start(
                                        dst[:, :, 0:1],
                                        zerbf_d[0:16, 0:NR // 2].rearrange(
                                            "p (r c) -> p r c", c=1))

                        for i in range(NCH):
                            c1 = psB.tile([128, 512], f32, tag="c1s", bufs=2)
                            nc.tensor.matmul(c1[:], w1s[:],
                                             im1[:, 512 * i:512 * (i + 1)],
                                             start=True, stop=True)
                            nc.scalar.activation(
                                a1s[:, :].rearrange("p (r c) -> p r c", c=RW)
                                [:, 2 * i:2 * i + 2, 1:257],
                                c1[:].rearrange("p (r c) -> p r c", r=2),
                                AF.Silu, bias=b1s[:])
                        nc.sync.dma_start(
                            a1s[:].rearrange("p (r c) -> p r c", c=RW)[:, :, 0:1],
                            zerbf_d[:, 0:R1].rearrange("p (r c) -> p r c", c=1))
                        nc.sync.dma_start(
                            a1s[:].rearrange("p (r c) -> p r c", c=RW)
                            [:, :, 257:258],
                            zerbf_d[:, 0:R1].rearrange("p (r c) -> p r c", c=1))
                        if s == 0:
                            nc.sync.dma_start(a1s[:, 1:257], zerbf_d[:, 0:256])
                        if s == NSTRIP - 1:
                            nc.sync.dma_start(
                                a1s[:, (R1 - 1) * RW + 1:(R1 - 1) * RW + 257],
                                zerbf_d[:, 0:256])

                        # ---- conv2 narrow -> z ----
                        z_fl = pE.tile([16, NPX2], f32r, tag="z", bufs=2)
                        for q in range(NQ):
                            c2n = psB.tile([16, 512], f32, tag="c2n",
                                           bufs=2)
                            nc.tensor.matmul(c2n[:], w2nA[:],
                                             im2A[:, 512 * q:512 * (q + 1)],
                                             start=True, stop=False)
                            nc.tensor.matmul(c2n[:], w2nB[:],
                                             im2B[:, 512 * q:512 * (q + 1)],
                                             start=False, stop=True)
                            nc.scalar.activation(z_fl[:, 512 * q:512 * (q + 1)],
                                                 c2n[:], AF.Silu, bias=b2n[:])

                        # ---- conv2 wide + silu + transpose + logits ----
                        ps_log = psA.tile([128, 32 * 4 * NQ], f32, tag="pslog")
                        for q in range(NQ):
                            c2 = psC2.tile([128, 512], f32, tag="c2s")
                            for t9 in range(9):
                                ky, kx = t9 // 3, t9 % 3
                                rhs = a1s[:, :].rearrange(
                                    "p (r c) -> p r c", c=RW)[
                                    :, 8 * q + ky: 8 * q + ky + 8: 2,
                                    kx: kx + 256: 2]
                                nc.tensor.matmul(
                                    c2[:], w2s[:, 128 * t9:128 * (t9 + 1)],
                                    rhs, start=(t9 == 0), stop=(t9 == 8))
                            ts_t = pst.tile([128, 512], bf16, tag="tst")
                            nc.scalar.activation(ts_t[:], c2[:], AF.Silu,
                                                 bias=b2s[:])
                            ps_tr = psB.tile([128, 512], bf16, tag="pstr")
                            for j in range(4):
                                nc.tensor.transpose(
                                    ps_tr[:, 128 * j:128 * (j + 1)],
                                    ts_t[:, 128 * j:128 * (j + 1)], identb[:])
                            nc.vector.tensor_copy(
                                tstT_all[:, 2048 * s + 512 * q:
                                         2048 * s + 512 * (q + 1)], ps_tr[:])
                            for j in range(4):
                                nc.tensor.matmul(
                                    ps_log[:, 32 * (4 * q + j):
                                           32 * (4 * q + j) + 32],
                                    z_fl[0:16, 512 * q + 128 * j:
                                         512 * q + 128 * (j + 1)],
                                    ckt[:], start=True, stop=True)
                        nc.vector.tensor_copy(
                            logit_all[:, 512 * s:512 * (s + 1)], ps_log[:])

                # ============ MEMCELL (batched softmax + AG + dV) ============
                with (
                    tc.tile_pool(name="pM", bufs=1) as pM,
                    tc.tile_pool(name="psM", bufs=1, space="PSUM") as psM,
                ):
                    # quartered softmax+A|G pipeline: ACT exp -> DVE
                    # reduce/recip/mult -> PE A|G, overlapped across quarters
                    e_all = pM.tile([128, 4096], f32)
                    den = pM.tile([128, 128], f32)
                    rec = pM.tile([128, 128], f32)
                    ps_ag = psM.tile([32, 160], f32)
                    for qq in range(4):
                        sl = slice(1024 * qq, 1024 * (qq + 1))
                        cs = slice(32 * qq, 32 * qq + 32)
                        nc.scalar.activation(e_all[:, sl], logit_all[:, sl],
                                             AF.Exp)
                        nc.vector.tensor_reduce(
                            den[:, cs],
                            e_all[:, sl].rearrange("p (c k) -> p c k", k=32),
                            mybir.AxisListType.X, ALU.add)
                        nc.vector.reciprocal(rec[:, cs], den[:, cs])
                        nc.vector.tensor_tensor(
                            w_bf[:, sl].rearrange("p (c k) -> p c k", k=32),
                            e_all[:, sl].rearrange("p (c k) -> p c k", k=32),
                            rec[:, cs].rearrange("p (c k) -> p c k", k=1)
                            .broadcast_to([128, 32, 32]),
                            ALU.mult)
                        for c in range(32 * qq, 32 * qq + 32):
                            lhs = w_bf[:, 32 * c:32 * c + 32]
                            nc.tensor.matmul(ps_ag[:, 0:128], lhs,
                                             tstT_all[:, 128 * c:128 * (c + 1)],
                                             start=(c == 0), stop=(c == 127))
                            nc.tensor.matmul(ps_ag[:, 128:160], lhs, lhs,
                                             start=(c == 0), stop=(c == 127))

                    # ---- dV + collective ----
                    a_sb = pM.tile([32, 128], f32)
                    nc.vector.tensor_copy(a_sb[:], ps_ag[:, 0:128])
                    g_sb = pM.tile([32, 32], f32r)
                    nc.vector.tensor_copy(g_sb[:], ps_ag[:, 128:160])
                    ps_gv_t = psM.tile([32, 128], f32, name="psgv")
                    nc.tensor.matmul(ps_gv_t[:], g_sb[:], vmat[:],
                                     start=True, stop=True)
                    dv_sb = pM.tile([32, 128], f32)
                    nc.vector.tensor_sub(dv_sb[:], a_sb[:], ps_gv_t[:])
                    dv_in = pdram.tile([32, 128], f32)
                    dv_out = pdram.tile([32 * N_CORES, 128], f32)
                    nc.sync.dma_start(dv_in[:], dv_sb[:])
                    if sc_sim:
                        for _c in range(N_CORES):
                            nc.sync.dma_start(
                                dv_out[32 * _c:32 * (_c + 1), :], dv_in[:])
                    else:
                        nc.gpsimd.collective_compute(
                            "AllGather", ALU.bypass,
                            replica_groups=[list(range(N_CORES))],
                            ins=[dv_in.opt()], outs=[dv_out.opt()])
                    gath = pM.tile([32, 8 * 128], f32)
                    nc.sync.dma_start(
                        gath[:].rearrange("p (r c) -> p r c", r=N_CORES),
                        dv_out[:].rearrange("(r p) c -> p r c", p=32))
                    nc.vector.tensor_add(gath[:, 0:512], gath[:, 0:512],
                                         gath[:, 512:1024])
                    nc.vector.tensor_add(gath[:, 0:256], gath[:, 0:256],
                                         gath[:, 256:512])
                    nc.vector.tensor_add(gath[:, 0:128], gath[:, 0:128],
                                         gath[:, 128:256])
                    nc.vector.scalar_tensor_tensor(
                        vnew_bf[:], gath[:, 0:128], ALPHA, vmat[:],
                        op0=ALU.mult, op1=ALU.add)

            # =====================  DECODER  =====================
            with (
                tc.tile_pool(name="pDimg", bufs=1) as pDimg,
                tc.tile_pool(name="pst2", bufs=2) as pst2,
                tc.tile_pool(name="psD", bufs=2, space="PSUM") as psD,
            ):
                _pDB_cm = tc.tile_pool(name="pDB", bufs=1)
                pDB = _pDB_cm.__enter__()
                _pDA_cm = tc.tile_pool(name="pDA", bufs=1)
                pDA = _pDA_cm.__enter__()
                # halves: A = padded rows 0..hiA, B = rows 64..129 (local -64)
                w_sT = pDA.tile([32, 16384], bf16)
                d0mA = pDA.tile([128, 69 * 130], f32r)
                d0mB = pDB.tile([128, 66 * 130], f32r)
                img12A = pDimg.tile([12, 66 * 130], f32r)
                img12B = pDimg.tile([12, 66 * 130], f32r)

                # ---- w slot-major via PE transpose (ACT does psum->sbuf) ----
                for g in range(32):           # 4 chunks per psum bank
                    ps_wt = psD.tile([32, 512], bf16, tag="ps_wt")
                    for j in range(4):
                        c = 4 * g + j
                        nc.tensor.transpose(ps_wt[:, 128 * j:128 * (j + 1)],
                                            w_bf[:, 32 * c:32 * c + 32],
                                            identb[:])
                    nc.scalar.activation(w_sT[:, 512 * g:512 * (g + 1)],
                                         ps_wt[:], AF.Copy)

                # ---- d0m pads ----
                for dm, nr, top, bot in ((d0mA, 69, True, False),
                                         (d0mB, 66, False, True)):
                    if top:
                        nc.sync.dma_start(dm[:, 0:130], zer_d[:, 0:130])
                    if bot:
                        nc.sync.dma_start(dm[:, (nr - 1) * 130:nr * 130],
                                          zer_d[:, 0:130])
                    nc.sync.dma_start(
                        dm[:].rearrange("p (r c) -> p r c", c=130)[:, :, 0:1],
                        zer_d[:, 0:nr].rearrange("p (r c) -> p r c", c=1))
                    nc.sync.dma_start(
                        dm[:].rearrange("p (r c) -> p r c", c=130)
                        [:, :, 129:130],
                        zer_d[:, 0:nr].rearrange("p (r c) -> p r c", c=1))

                # ---- t_read -> d0m halves ----
                for q in range(32):
                    ps_rd = psD.tile([128, 512], f32, tag="psrd")
                    nc.tensor.matmul(ps_rd[:], vnew_bf[:],
                                     w_sT[:, 512 * q:512 * (q + 1)],
                                     start=True, stop=True)
                    rd4 = ps_rd[:].rearrange("p (r c) -> p r c", r=4)
                    if q <= 16:
                        nc.vector.tensor_copy(
                            d0mA[:].rearrange("p (r c) -> p r c", c=130)
                            [:, 4 * q + 1:4 * q + 5, 1:129], rd4)
                    if q == 15:   # global row 64 -> B local 0
                        nc.vector.tensor_copy(
                            d0mB[:].rearrange("p (r c) -> p r c", c=130)
                            [:, 0:1, 1:129], rd4[:, 3:4, :])
                    if q >= 16:
                        lr = 4 * q + 1 - 64
                        nc.vector.tensor_copy(
                            d0mB[:].rearrange("p (r c) -> p r c", c=130)
                            [:, lr:lr + 4, 1:129], rd4)

                # ---- img12 pads (row pads; sides) ----
                nc.sync.dma_start(img12A[:, 0:130], zer_d[0:12, 0:130])
                nc.sync.dma_start(img12B[:, 65 * 130:66 * 130],
                                  zer_d[0:12, 0:130])
                for im in (img12A, img12B):
                    nc.sync.dma_start(
                        im[:].rearrange("p (r c) -> p r c", c=130)[:, :, 0:1],
                        zer_d[0:12, 0:66].rearrange("p (r c) -> p r c", c=1))
                    nc.sync.dma_start(
                        im[:].rearrange("p (r c) -> p r c", c=130)
                        [:, :, 129:130],
                        zer_d[0:12, 0:66].rearrange("p (r c) -> p r c", c=1))

                # ---- deconv -> silu -> img12 halves ----
                UC = [(1, -1), (0, 0), (1, 0), (0, 1)]   # (parity, shift)

                c3t = {}

                def _emit_imc(h, gps=(0, 1)):
                    imcs = c3t['imcs']
                    imh = img12A if h == 0 else img12B
                    nd = 0
                    for gp in gps:
                        for ia, (pa, dya) in enumerate(UC):
                            for ib, (pb, dxb) in enumerate(UC):
                                eng = nc.scalar if nd % 2 == 0 else nc.sync
                                nd += 1
                                eng.dma_start(
                                    imcs[h][48 * gp + 12 * ia + 3 * ib:
                                            48 * gp + 12 * ia + 3 * ib + 3, :]
                                    .rearrange("p (r c) -> p r c", c=128),
                                    imh[:].rearrange("p (r c) -> p r c", c=130)
                                    [6 * pa + 3 * pb:6 * pa + 3 * pb + 3,
                                     32 * gp + dya + 1:32 * gp + dya + 33,
                                     dxb + 1:dxb + 129])
                for q in range(32):
                    dm = d0mA if q <= 15 else d0mB
                    base = 4 * q + 1 if q <= 15 else 4 * q + 1 - 64
                    ps_dec = psD.tile([12, 512], f32, tag="psdec")
                    for t9 in range(9):
                        dy, dx = t9 // 3 - 1, t9 % 3 - 1
                        rhs = dm[:].rearrange("p (r c) -> p r c", c=130)[
                            :, base + dy:base + 4 + dy, 1 + dx:129 + dx]
                        nc.tensor.matmul(ps_dec[:],
                                         decw[:, 12 * t9:12 * (t9 + 1)],
                                         rhs, start=(t9 == 0), stop=(t9 == 8))
                    im = img12A if q <= 15 else img12B
                    nc.scalar.activation(
                        _mk_ap(im[:], base * 130 + 1, [[130, 4], [1, 128]]),
                        ps_dec[:].rearrange("p (r c) -> p r c", r=4),
                        AF.Silu, bias=bdec[:])
                    if q == 15:   # dup A row 64 -> B local 0
                        nc.gpsimd.tensor_copy(img12B[:, 0:130],
                                              img12A[:, 64 * 130:65 * 130])
                    if q == 16:   # dup B local 1 (g65) -> A row 65
                        nc.gpsimd.tensor_copy(img12A[:, 65 * 130:66 * 130],
                                              img12B[:, 130:260])
                        # d0mA/w_sT dead; alloc conv3 tiles, start imc h=0
                        _pDA_cm.__exit__(None, None, None)
                        _pD2_cm = tc.tile_pool(name="pD2", bufs=1)
                        pD2 = _pD2_cm.__enter__()
                        c3t['imcs'] = [
                            pD2.tile([96, 4096], f32r, name=f"imc{hh}")
                            for hh in range(2)]
                        c3t['rbp'] = [
                            pD2.tile([6, 8192], bf16, name=f"rb{a_}")
                            for a_ in range(2)]
                        _emit_imc(0)

                # ---- conv3 per half: union im2col [96, 4096] ----
                # imcH[48g'+12ia+3ib+c, 128rh+J'] =
                #   img12H[6a+3b+c, 32g'+rh+dy+1, J'+dx+1]
                imcs = c3t['imcs']
                rbs = [c3t['rbp'], c3t['rbp']]
                outv = out_d[0:3, :, :].rearrange(
                    "p (hh gp ch r a) c -> p hh gp ch r a c",
                    hh=2, gp=2, ch=8, r=4, a=2)
                for h in range(2):
                    if h == 1:
                        _emit_imc(1)
                    for ch in range(8):
                        c3 = psD.tile([128, 512], f32, tag="c3")
                        nc.tensor.matmul(
                            c3[:], lhs48[:],
                            imcs[h][:, 512 * ch:512 * (ch + 1)],
                            start=True, stop=True)
                        stg = pst2.tile([128, 512], bf16, tag="stg")
                        nc.scalar.activation(stg[:], c3[:], AF.Silu,
                                             bias=b3r[:])
                        for cl in range(4):
                            a_, b_ = cl // 2, cl % 2
                            nc.vector.tensor_copy(
                                _mk_ap(rbs[h][a_][:], 1024 * ch + b_,
                                       [[256, 4], [2, 128]]),
                                stg[32 * cl:32 * cl + 6, :]
                                .rearrange("p (r c) -> p r c", r=4))
                    for a_ in range(2):
                        for gp in range(2):
                            nc.gpsimd.dma_start(
                                outv[:, h, gp, :, :, a_, :],
                                rbs[h][a_][3 * gp:3 * gp + 3, :]
                                .rearrange("p (ch r c) -> p ch r c",
                                           ch=8, r=4))
                _pD2_cm.__exit__(None, None, None)
                _pDB_cm.__exit__(None, None, None)

    nc.compile()
    return nc

def _prep_weights(i):
    """Host-side weight layout prep. i = dict of full inputs."""
    f = np.float32
    w1s = np.ascontiguousarray(
        i['e0s_w1'].transpose(2, 3, 1, 0).reshape(27, 128)).astype(BF)
    w1n27 = np.ascontiguousarray(
        i['e0n_w1'].transpose(2, 3, 1, 0).reshape(27, 16)).astype(f)
    w1n4 = np.zeros((108, 64), f)
    for g in range(4):
        w1n4[27 * g:27 * g + 27, 16 * g:16 * g + 16] = w1n27
    w2s = np.ascontiguousarray(
        i['e0s_w2'].transpose(1, 2, 3, 0).reshape(128, 9 * 128)).astype(BF)
    w2n = np.ascontiguousarray(
        i['e0n_w2'].transpose(2, 3, 1, 0).reshape(9, 16, 16)).astype(f)
    w2nA = w2n[0:8].reshape(128, 16).astype(BF).copy()
    w2nB = w2n[8].astype(BF).copy()
    ckt = (i['cell_k'].T * np.float32(0.25)).astype(f).copy()   # /sqrt(16)
    vmat = i['cell_v'].astype(f).copy()
    # deconv: shift s=(dy,dx); decw[s][c, (a*2+b)*3+o] = W[c,o,ky(a,u),kx(b,v)]
    dw = i['d0_dw']  # [128, 3, 4, 4]
    decw = np.zeros((9, 128, 12), f)
    for a in range(2):
        for u in range(2):
            ky = (1, 3)[u] if a == 0 else (0, 2)[u]
            dy = (0, -1)[u] if a == 0 else (1, 0)[u]
            for b in range(2):
                for v in range(2):
                    kx = (1, 3)[v] if b == 0 else (0, 2)[v]
                    dx = (0, -1)[v] if b == 0 else (1, 0)[v]
                    sidx = (dy + 1) * 3 + (dx + 1)
                    for o in range(3):
                        decw[sidx, :, (a * 2 + b) * 3 + o] += dw[:, o, ky, kx]
    # conv3 union-packed lhs: row 48g+12ia+3ib+c, col 6*(2a'+b')+3g+o
    UC = [(1, -1), (0, 0), (1, 0), (0, 1)]
    cw = i['d0_cw']  # [3 out, 3 in, 3, 3]
    lhs48 = np.zeros((96, 128), f)
    for ap_ in range(2):
        for bp in range(2):
            cl = 2 * ap_ + bp
            for g in range(2):
                for ia, (pa, dya) in enumerate(UC):
                    ky = 2 * dya + pa + 1 - ap_
                    if not (0 <= ky < 3):
                        continue
                    for ib, (pb, dxb) in enumerate(UC):
                        kx = 2 * dxb + pb + 1 - bp
                        if not (0 <= kx < 3):
                            continue
                        for c in range(3):
                            for o in range(3):
                                lhs48[48 * g + 12 * ia + 3 * ib + c,
                                      32 * cl + 3 * g + o] = cw[o, c, ky, kx]
    bdec = np.zeros((12, 1), f)
    for ab in range(4):
        bdec[3 * ab:3 * ab + 3, 0] = i['d0_db']
    b3r = np.zeros((128, 1), f)
    for p in range(128):
        b3r[p, 0] = i['d0_cb'][(p % 32) % 3]
    return dict(
        w1s=w1s, w1n4=w1n4.astype(BF), w2s=w2s, w2nA=w2nA, w2nB=w2nB, ckt=ckt, vmat=vmat,
        decw=np.ascontiguousarray(
            decw.transpose(1, 0, 2).reshape(128, 108)).astype(f),
        lhs48=lhs48.astype(f),
        b1s=i['e0s_b1'].reshape(128, 1).astype(f),
        b1n=np.tile(i['e0n_b1'].reshape(16, 1), (4, 1)).astype(f),
        b2s=i['e0s_b2'].reshape(128, 1).astype(f),
        b2n=i['e0n_b2'].reshape(16, 1).astype(f),
        bdec=bdec, b3r=b3r,
        zer=np.zeros((128, 1024), f),
        zerbf=np.zeros((128, 512), BF),
        zer27=np.zeros((27, 34 * 256), BF),
    )


_last = {}


def last_exec_ns():
    return _last.get('ns')


def _get_runner():
    """Cached jitted SPMD callable over 8 cores (traced once)."""
    if 'runner' in _cache:
        return _cache['runner']
    import jax
    from jax.sharding import Mesh, PartitionSpec
    from jax.experimental.shard_map import shard_map
    from concourse import bass2jax, mybir as _mb
    nc = _cache['nc']
    bass2jax.install_neuronx_cc_hook()
    partition_name = nc.partition_id_tensor.name if nc.partition_id_tensor else None
    in_names, out_names, out_avals, zero_outs = [], [], [], []
    for alloc in nc.m.functions[0].allocations:
        if not isinstance(alloc, _mb.MemoryLocationSet):
            continue
        name = alloc.memorylocations[0].name
        if alloc.kind == "ExternalInput":
            if name != partition_name:
                in_names.append(name)
        elif alloc.kind == "ExternalOutput":
            shape = tuple(alloc.tensor_shape)
            dtype = _mb.dt.np(alloc.dtype)
            out_names.append(name)
            out_avals.append(jax.core.ShapedArray(shape, dtype))
            zero_outs.append(np.zeros(shape, dtype))
    n_params = len(in_names)
    n_outs = len(out_avals)
    all_names = list(in_names) + list(out_names)
    if partition_name is not None:
        all_names.append(partition_name)

    def _body(*args):
        operands = list(args)
        if partition_name is not None:
            operands.append(bass2jax.partition_id_tensor())
        outs = bass2jax._bass_exec_p.bind(
            *operands, out_avals=tuple(out_avals), in_names=tuple(all_names),
            out_names=tuple(out_names), lowering_input_output_aliases=(),
            sim_require_finite=True, sim_require_nnan=True, nc=nc)
        return tuple(outs)

    devices = jax.devices()[:N_CORES]
    mesh = Mesh(np.asarray(devices), ("core",))
    sharded = jax.jit(
        shard_map(_body, mesh=mesh,
                  in_specs=(PartitionSpec("core"),) * (n_params + n_outs),
                  out_specs=(PartitionSpec("core"),) * n_outs,
                  check_rep=False),
        keep_unused=True)

    from jax.sharding import NamedSharding
    sh = NamedSharding(mesh, PartitionSpec("core"))
    _cache['sharding'] = sh
    _cache['devices'] = devices
    _cache['runner'] = (sharded, in_names, out_names, out_avals, zero_outs)
    return _cache['runner']


def _make_global(per_core_arrs):
    """Assemble a sharded global array from per-core numpy shards."""
    import jax
    sh = _cache['sharding']
    devices = _cache['devices']
    a0 = np.asarray(per_core_arrs[0])
    global_shape = (len(per_core_arrs) * a0.shape[0], *a0.shape[1:])
    bufs = [jax.device_put(np.ascontiguousarray(a), d)
            for a, d in zip(per_core_arrs, devices)]
    return jax.make_array_from_single_device_arrays(global_shape, sh, bufs)


def _run_fast(in_maps):
    import jax
    sharded, in_names, out_names, out_avals, zero_outs = _get_runner()
    # fresh output buffers every call: cached ones may have been donated
    _cache['dev_zeros'] = [
        _make_global([np.zeros(z.shape, z.dtype)] * N_CORES)
        for z in zero_outs]
    n_cores = len(in_maps)
    gin = [_make_global([in_maps[c][nm] for c in range(n_cores)])
           for nm in in_names]
    outs = sharded(*gin, *_cache['dev_zeros'])
    return [{nm: np.asarray(outs[i]).reshape(n_cores, *out_avals[i].shape)[c]
             for i, nm in enumerate(out_names)} for c in range(n_cores)]


def _build_tiny():
    nc = bacc.Bacc("TRN2", target_bir_lowering=False, name="tiny")
    xi = nc.dram_tensor("xi", [128, 128], f32, kind="ExternalInput")
    xo = nc.dram_tensor("xo", [128, 128], f32, kind="ExternalOutput")
    with tile.TileContext(nc) as tc:
        with tc.tile_pool(name="sb", bufs=1) as sb:
            t = sb.tile([128, 128], f32)
            nc.sync.dma_start(t[:], xi[:])
            nc.sync.dma_start(xo[:], t[:])
    nc.compile()
    return nc


def bench_hw(n_iter=12, **inputs):
    """Estimate device exec time: full-kernel min wall minus trivial-kernel
    min wall (same 8-core dispatch path)."""
    import time as _t, jax
    from jax.sharding import Mesh, PartitionSpec
    from jax.experimental.shard_map import shard_map
    from concourse import bass2jax
    if 'nc' not in _cache:
        _cache['nc'] = _build()
    shared = _prep_weights({k: np.asarray(v) for k, v in inputs.items()})
    x = np.asarray(inputs['x'], dtype=np.float32)
    in_maps = [dict(shared, x=np.ascontiguousarray(x[c])) for c in range(N_CORES)]
    sharded, in_names, out_names, out_avals, zero_outs = _get_runner()
    gin = [_make_global([in_maps[c][nm] for c in range(N_CORES)])
           for nm in in_names]
    gz = [_make_global([np.zeros(z.shape, z.dtype)] * N_CORES)
          for z in zero_outs]

    def mintime(fn, args):
        ts = []
        for _ in range(n_iter):
            t0 = _t.perf_counter()
            o = fn(*args)
            jax.block_until_ready(o)
            ts.append(_t.perf_counter() - t0)
        return min(ts), ts

    tfull, ts_full = mintime(sharded, (*gin, *gz))

    if 'tiny_fn' not in _cache:
        ncT = _build_tiny()
        bass2jax.install_neuronx_cc_hook()
        pn = ncT.partition_id_tensor.name if ncT.partition_id_tensor else None

        def _tb(xi, xoz):
            ops = [xi, xoz]
            if pn is not None:
                ops.append(bass2jax.partition_id_tensor())
            names = ["xi", "xo"] + ([pn] if pn else [])
            return tuple(bass2jax._bass_exec_p.bind(
                *ops,
                out_avals=(jax.core.ShapedArray((128, 128), np.float32),),
                in_names=tuple(names), out_names=("xo",),
                lowering_input_output_aliases=(),
                sim_require_finite=True, sim_require_nnan=True, nc=ncT))
        mesh = Mesh(np.asarray(_cache['devices']), ("core",))
        _cache['tiny_fn'] = jax.jit(shard_map(
            _tb, mesh=mesh, in_specs=(PartitionSpec("core"),) * 2,
            out_specs=(PartitionSpec("core"),), check_rep=False),
            keep_unused=True)
        _cache['tiny_in'] = (
            _make_global([np.zeros((128, 128), np.float32)] * N_CORES),
            _make_global([np.zeros((128, 128), np.float32)] * N_CORES))
    ttiny, ts_tiny = mintime(_cache['tiny_fn'], _cache['tiny_in'])
    return max(0.0, tfull - ttiny), tfull, ttiny


def bench(n_iter=20, **inputs):
    """Min wall time of the on-device executable (inputs pre-staged)."""
    import time as _t, jax
    if 'nc' not in _cache:
        _cache['nc'] = _build()
    shared = _prep_weights({k: np.asarray(v) for k, v in inputs.items()})
    x = np.asarray(inputs['x'], dtype=np.float32)
    in_maps = [dict(shared, x=np.ascontiguousarray(x[c])) for c in range(N_CORES)]
    sharded, in_names, out_names, out_avals, zero_outs = _get_runner()
    if 'dev_zeros' not in _cache:
        _cache['dev_zeros'] = [
            _make_global([np.zeros(z.shape, z.dtype)] * N_CORES)
            for z in zero_outs]
    gin = [_make_global([in_maps[c][nm] for c in range(N_CORES)])
           for nm in in_names]
    times = []
    for it in range(n_iter):
        t0 = _t.perf_counter()
        outs = sharded(*gin, *_cache['dev_zeros'])
        jax.block_until_ready(outs)
        times.append(_t.perf_counter() - t0)
    return min(times), times


def kernel(**inputs):
    # Re-executing the same loaded program leaves device state that breaks
    # warm runs; rebuild a fresh program (new executable/load) per call.
    if _cache.get('ran_once'):
        _cache.clear()
    if 'nc' not in _cache:
        _cache['nc'] = _build()
    _cache['ran_once'] = True
    nc = _cache['nc']
    shared = _prep_weights({k: np.asarray(v) for k, v in inputs.items()})
    x = np.asarray(inputs['x'], dtype=np.float32)
    in_maps = [dict(shared, x=np.ascontiguousarray(x[c])) for c in range(N_CORES)]
    res = _run_fast(in_maps)
    out = np.stack([res[c]["out"] for c in range(N_CORES)], axis=0)
    return out
